# revision 9
# baseline (speedup 1.0000x reference)
"""Trainium2 Bass kernel for nn_Discriminator (GNN message passing).

Math (reference):
    h   = relu(embedding @ W_emb + b_emb)          # [N, HID]
    w_sym = 0.5*(W_edge[:HID,0] + W_edge[HID:,0])  # [HID]
    raw = (h[e0] + h[e1]) @ w_sym + b_edge         # [E]
    out = sigmoid(logit(eps) + raw),  eps = (2B-1)*u + (1-B)

Key algebraic reduction: raw[e] = s[e0] + s[e1] + b_edge with the per-node
scalar s = h @ w_sym, so the edge stage only gathers scalars from a
50k-entry table instead of [E, HID] rows.

Distribution (8 NeuronCores):
  - node GEMM sharded over N (6250 nodes/core, padded 6272)
  - AllGather of s ([6272] f32 per core -> [50176])
  - edge stage data-parallel over E (100000 edges/core, padded 100352)

Edge gather on-core: the s table is laid out 16-way chunked across each
GPSIMD core's partitions (partition p holds s[3136*(p%16) : +3136]); an
ap_gather with wrapped int16 in-chunk offsets returns candidate values in
all 16 partitions; a scalar_tensor_tensor (is_equal x mult) masks the
correct chunk and a PE matmul with a group-indicator reduces over the 16
partitions, accumulating both edge endpoints into one PSUM tile.
"""

import os
import sys
import types
import contextlib
import ctypes

sys.path.insert(0, "/opt/trn_rl_repo")

import numpy as np

import concourse.bass as bass
import concourse.mybir as mybir
import concourse.tile as tile
import concourse.bacc as bacc
from concourse.bass_utils import run_bass_kernel_spmd

# ---------------------------------------------------------------- constants
N, IN_DIM, HID, E = 50000, 512, 256, 800000
NCORES = 8
BIAS = 0.0001

NLOC = N // NCORES          # 6250 real nodes per core
NLOC_PAD = 6272             # 49*128, padded local nodes (GEMM)
SLABS = 13                  # s staging rows of 512 (13*512 = 6656)
RANK_PAD = SLABS * 512      # 6656 s-values shipped per rank (tail is garbage)
NPAD = RANK_PAD * NCORES    # 53248 = 16*3328
CHUNK = NPAD // 16          # 3328 table entries per partition (<= 32768, int16-safe)
ELOC = E // NCORES          # 100000 edges per core
EG = ELOC // 8              # 12500 edges per gpsimd core-group
EG_PAD = 12544              # 16*784
F = EG_PAD // 16            # 784
ELOC_PAD = EG_PAD * 8       # 100352
GCALL = 3136                # indices per ap_gather call (4 calls per endpoint)
WIN = 448                   # select/reduce window (3136 = 7*448)

f32 = mybir.dt.float32
f32r = mybir.dt.float32r


def _install_ntff_hook():
    """Provide antenv.axon_hooks (absent in this image) so trace=True works."""
    if "antenv.axon_hooks" in sys.modules:
        return
    try:
        lib = ctypes.CDLL("/opt/axon/libaxon_pjrt.so")
    except OSError:
        return
    if not hasattr(lib, "axon_start_nrt_profile"):
        return
    lib.axon_start_nrt_profile.argtypes = [ctypes.POINTER(ctypes.c_int64), ctypes.c_size_t]
    lib.axon_start_nrt_profile.restype = ctypes.c_int64
    lib.axon_stop_nrt_profile.argtypes = [ctypes.c_char_p]
    lib.axon_stop_nrt_profile.restype = ctypes.c_int64

    @contextlib.contextmanager
    def _hook(output_dir, device_ids):
        import jax
        jax.devices()
        if device_ids:
            ids = (ctypes.c_int64 * len(device_ids))(*device_ids)
            rc = lib.axon_start_nrt_profile(ids, len(device_ids))
        else:
            rc = lib.axon_start_nrt_profile(None, 0)
        if rc != 0:
            raise RuntimeError(f"axon_start_nrt_profile rc={rc}")
        try:
            yield
        finally:
            n = lib.axon_stop_nrt_profile(str(output_dir).encode())
            print(f"profile: {n} file(s) written to {output_dir}", file=sys.stderr)

    mod = types.ModuleType("antenv.axon_hooks")
    mod.get_axon_ntff_profile_hook = lambda: _hook
    mod.set_axon_ntff_profile_hook = lambda h: None
    sys.modules["antenv.axon_hooks"] = mod


_install_ntff_hook()

_PROGRAM_CACHE = {}


def _finish(nc):
    return nc


def _build_program(debug: bool, stage: int = 5):
    nc = _build_body(debug, stage)
    nc.finalize()
    return nc


def _build_body(debug: bool, stage: int = 5):
    nc = bacc.Bacc(None)

    embT = nc.dram_tensor("embT", [IN_DIM, NLOC_PAD], f32, kind="ExternalInput")
    Wt = nc.dram_tensor("Wt", [IN_DIM, HID], f32, kind="ExternalInput")
    bias2 = nc.dram_tensor("bias2", [128, 2], f32, kind="ExternalInput")
    wsym2 = nc.dram_tensor("wsym2", [128, 2], f32, kind="ExternalInput")
    bedge = nc.dram_tensor("bedge", [128, 3], f32, kind="ExternalInput")
    iota16 = nc.dram_tensor("iota16", [128, 1], f32, kind="ExternalInput")
    e8 = nc.dram_tensor("e8", [128, 8], f32, kind="ExternalInput")
    r0w = nc.dram_tensor("r0w", [128, F], mybir.dt.int16, kind="ExternalInput")
    r1w = nc.dram_tensor("r1w", [128, F], mybir.dt.int16, kind="ExternalInput")
    c0u = nc.dram_tensor("c0u", [128, EG_PAD], mybir.dt.uint8, kind="ExternalInput")
    c1u = nc.dram_tensor("c1u", [128, EG_PAD], mybir.dt.uint8, kind="ExternalInput")
    uu = nc.dram_tensor("uu", [128, F], f32, kind="ExternalInput")
    out = nc.dram_tensor("out", [128, F], f32, kind="ExternalOutput")
    if debug:
        sdbg = nc.dram_tensor("sdbg", [16, CHUNK], f32, kind="ExternalOutput")
        rawdbg = nc.dram_tensor("rawdbg", [128, F], f32, kind="ExternalOutput")

    with tile.TileContext(nc) as tc:
        with (
            tc.tile_pool(name="const", bufs=1) as constp,
            tc.tile_pool(name="w", bufs=1) as wp,
            tc.tile_pool(name="emb", bufs=3) as embp,
            tc.tile_pool(name="h", bufs=2) as hp,
            tc.tile_pool(name="s", bufs=2) as sp,
            tc.tile_pool(name="tab", bufs=1) as tabp,
            tc.tile_pool(name="gat", bufs=2) as gatp,
            tc.tile_pool(name="win", bufs=3) as winp,
            tc.tile_pool(name="fin", bufs=1) as finp,
            tc.tile_pool(name="psA", bufs=2, space="PSUM") as psA,
            tc.tile_pool(name="psS", bufs=2, space="PSUM") as psS,
            tc.tile_pool(name="psR", bufs=2, space="PSUM") as psR,
            tc.tile_pool(name="dram", bufs=1, space="DRAM") as dramp,
        ):
            # ---------------- constants into SBUF
            t_bias2 = constp.tile([128, 2], f32)
            nc.sync.dma_start(t_bias2[:], bias2[:])
            t_wsym2 = constp.tile([128, 2], f32)
            nc.sync.dma_start(t_wsym2[:], wsym2[:])
            t_bedge = constp.tile([128, 3], f32)
            nc.sync.dma_start(t_bedge[:], bedge[:])
            t_iota16 = constp.tile([128, 1], f32)
            nc.sync.dma_start(t_iota16[:], iota16[:])
            t_e8 = constp.tile([128, 8], f32)
            nc.sync.dma_start(t_e8[:], e8[:])
            t_W = wp.tile([128, 4 * HID], f32)  # k-chunk k at [:, k*HID:(k+1)*HID]
            for k in range(4):
                nc.sync.dma_start(t_W[:, k * HID:(k + 1) * HID], Wt[128 * k:128 * (k + 1), :])

            # ---------------- stage A: s = relu(emb @ W + b) @ w_sym
            d_sin = dramp.tile([SLABS, 512], f32)
            slabs = []
            off = 0
            while off < NLOC_PAD:
                w = min(512, NLOC_PAD - off)
                slabs.append((off, w))
                off += w
            for (off, w) in slabs:
                t_embs = embp.tile([128, 4 * 512], f32, tag="embs")
                for k in range(4):
                    nc.sync.dma_start(
                        t_embs[:, k * 512:k * 512 + w],
                        embT[128 * k:128 * (k + 1), off:off + w],
                    )
                ps_s = psS.tile([1, 512], f32, tag="ps_s")
                for H in range(2):
                    ps_h = psA.tile([128, 512], f32, tag="ps_h")
                    for k in range(4):
                        nc.tensor.matmul(
                            ps_h[:, :w],
                            lhsT=t_W[:, k * HID + 128 * H:k * HID + 128 * (H + 1)],
                            rhs=t_embs[:, k * 512:k * 512 + w],
                            start=(k == 0),
                            stop=(k == 3),
                        )
                    t_h = hp.tile([128, 512], f32, tag="h")
                    nc.scalar.activation(
                        t_h[:, :w], ps_h[:, :w],
                        mybir.ActivationFunctionType.Relu,
                        bias=t_bias2[:, H:H + 1],
                    )
                    nc.tensor.matmul(
                        ps_s[:1, :w],
                        lhsT=t_wsym2[:, H:H + 1],
                        rhs=t_h[:, :w],
                        start=(H == 0),
                        stop=(H == 1),
                    )
                t_sst = sp.tile([1, 512], f32, tag="sst")
                nc.vector.tensor_copy(t_sst[:1, :w], ps_s[:1, :w])
                if w < 512:
                    nc.vector.memset(t_sst[:1, w:512], 0.0)
                nc.sync.dma_start(d_sin[off // 512:off // 512 + 1, :], t_sst[:1, :])

            # ---------------- stage B: AllGather s
            if stage < 2:
                t_z = finp.tile([128, F], f32)
                nc.gpsimd.memset(t_z[:], 0.0)
                nc.sync.dma_start(out[:, :], t_z[:])
                return _finish(nc)
            d_sout = dramp.tile([16, CHUNK], f32)
            nc.gpsimd.collective_compute(
                "AllGather",
                mybir.AluOpType.bypass,
                ins=[d_sin[:].opt()],
                outs=[d_sout[:].opt()],
                replica_groups=[list(range(NCORES))],
            )
            t_tab = tabp.tile([128, CHUNK], f32)
            for g in range(8):
                nc.sync.dma_start(t_tab[16 * g:16 * (g + 1), :], d_sout[:, :])
            if debug:
                nc.sync.dma_start(sdbg[:, :], d_sout[:, :])

            # ---------------- stage C/D: chunked gather + select + reduce
            if stage < 3:
                t_z = finp.tile([128, F], f32)
                nc.gpsimd.memset(t_z[:], 0.0)
                nc.sync.dma_start(out[:, :], t_z[:])
                return _finish(nc)
            t_r0 = constp.tile([128, F], mybir.dt.int16)
            nc.sync.dma_start(t_r0[:], r0w[:])
            t_r1 = constp.tile([128, F], mybir.dt.int16)
            nc.sync.dma_start(t_r1[:], r1w[:])

            d_rawsp = dramp.tile([8, EG_PAD], f32)
            tab3 = t_tab[:].rearrange("p (n d) -> p n d", d=1)
            ncall = EG_PAD // GCALL
            NW = GCALL // WIN
            for k in range(ncall):
                gs = []
                for (t_r, c_dram, etag) in ((t_r0, c0u, "0"), (t_r1, c1u, "1")):
                    t_g = gatp.tile([128, GCALL], f32, tag="g" + etag)
                    nc.gpsimd.ap_gather(
                        t_g[:].rearrange("p (n d) -> p n d", d=1),
                        tab3,
                        t_r[:, k * (GCALL // 16):(k + 1) * (GCALL // 16)],
                        channels=128,
                        num_elems=CHUNK,
                        d=1,
                        num_idxs=GCALL,
                    )
                    t_cu = gatp.tile([128, GCALL], mybir.dt.uint8, tag="c" + etag)
                    nc.sync.dma_start(t_cu[:], c_dram[:, k * GCALL:(k + 1) * GCALL])
                    gs.append((t_g, t_cu))
                for wi in range(NW):
                    lo = wi * WIN
                    ps_r = psR.tile([8, WIN], f32, tag="ps_r")
                    for ei, (t_g, t_cu) in enumerate(gs):
                        t_cf = winp.tile([128, WIN], f32, tag="cf")
                        nc.vector.tensor_copy(t_cf[:], t_cu[:, lo:lo + WIN])
                        t_m = winp.tile([128, WIN], f32, tag="m")
                        nc.vector.scalar_tensor_tensor(
                            t_m[:],
                            in0=t_cf[:],
                            scalar=t_iota16[:, 0:1],
                            in1=t_g[:, lo:lo + WIN],
                            op0=mybir.AluOpType.is_equal,
                            op1=mybir.AluOpType.mult,
                        )
                        nc.tensor.matmul(
                            ps_r[:],
                            lhsT=t_e8[:],
                            rhs=t_m[:],
                            start=(ei == 0),
                            stop=(ei == 1),
                        )
                    t_rw = winp.tile([8, WIN], f32, tag="rw")
                    nc.vector.tensor_copy(t_rw[:], ps_r[:])
                    nc.sync.dma_start(d_rawsp[:, k * GCALL + lo:k * GCALL + lo + WIN], t_rw[:])

            # fold [8, EG_PAD] -> [128, F]
            if stage < 4:
                t_z = finp.tile([128, F], f32)
                nc.gpsimd.memset(t_z[:], 0.0)
                nc.sync.dma_start(out[:, :], t_z[:])
                return _finish(nc)
            t_raw = finp.tile([128, F], f32)
            for g in range(8):
                nc.sync.dma_start(
                    t_raw[16 * g:16 * (g + 1), :],
                    d_rawsp[g, :].rearrange("(c f) -> c f", c=16),
                )
            if debug:
                nc.sync.dma_start(rawdbg[:, :], t_raw[:, :])

            # ---------------- stage E: logit(eps) + raw, sigmoid
            if stage < 5:
                nc.sync.dma_start(out[:, :], t_raw[:])
                return _finish(nc)
            t_u = finp.tile([128, F], f32)
            nc.sync.dma_start(t_u[:], uu[:])
            a = 1.0 - 2.0 * BIAS
            t_l1 = finp.tile([128, F], f32)
            nc.scalar.activation(t_l1[:], t_u[:], mybir.ActivationFunctionType.Ln,
                                 bias=t_bedge[:, 1:2], scale=-a)
            t_l2 = finp.tile([128, F], f32)
            nc.scalar.activation(t_l2[:], t_u[:], mybir.ActivationFunctionType.Ln,
                                 bias=t_bedge[:, 2:3], scale=a)
            t_gate = finp.tile([128, F], f32)
            nc.vector.tensor_sub(t_gate[:], t_l1[:], t_l2[:])
            t_gate2 = finp.tile([128, F], f32)
            nc.vector.tensor_add(t_gate2[:], t_gate[:], t_raw[:])
            t_out = finp.tile([128, F], f32)
            nc.scalar.activation(t_out[:], t_gate2[:], mybir.ActivationFunctionType.Sigmoid,
                                 bias=t_bedge[:, 0:1])
            nc.sync.dma_start(out[:, :], t_out[:])

    return nc


def _prep_inputs(embedding, edges, u, W_emb, b_emb, W_edge, b_edge):
    """Host-side sharding / layout prep. Returns per-core input maps."""
    embedding = np.ascontiguousarray(np.asarray(embedding, dtype=np.float32))
    edges = np.asarray(edges).astype(np.int64)
    u = np.asarray(u, dtype=np.float32)
    W_emb = np.asarray(W_emb, dtype=np.float32)
    b_emb = np.asarray(b_emb, dtype=np.float32)
    W_edge = np.asarray(W_edge, dtype=np.float32)
    b_edge = np.asarray(b_edge, dtype=np.float32)

    wsym = 0.5 * (W_edge[:HID, 0] + W_edge[HID:, 0])
    bias2 = b_emb.reshape(2, 128).T.copy()           # [128, 2]
    wsym2 = wsym.reshape(2, 128).T.copy()            # [128, 2]
    bedge = np.tile(np.array([[b_edge[0], 1.0 - BIAS, BIAS]], np.float32), (128, 1))
    iota16 = (np.arange(128) % 16).astype(np.float32)[:, None]
    e8 = (np.arange(128)[:, None] // 16 == np.arange(8)[None, :]).astype(np.float32)

    # remap node id -> padded id
    ip = RANK_PAD * (edges // NLOC) + (edges % NLOC)  # [2, E]

    in_maps = []
    for c in range(NCORES):
        embT = np.zeros((IN_DIM, NLOC_PAD), np.float32)
        embT[:, :NLOC] = embedding[NLOC * c:NLOC * (c + 1)].T

        sl = slice(ELOC * c, ELOC * (c + 1))
        i0 = ip[0, sl].reshape(8, EG)
        i1 = ip[1, sl].reshape(8, EG)
        uc = u[sl].reshape(8, EG)
        # pad per gpsimd group
        i0p = np.zeros((8, EG_PAD), np.int64); i0p[:, :EG] = i0
        i1p = np.zeros((8, EG_PAD), np.int64); i1p[:, :EG] = i1
        up = np.full((8, EG_PAD), 0.5, np.float32); up[:, :EG] = uc

        r0 = (i0p % CHUNK).astype(np.int16)
        r1 = (i1p % CHUNK).astype(np.int16)
        c0 = (i0p // CHUNK).astype(np.uint8)
        c1 = (i1p // CHUNK).astype(np.uint8)

        # wrap [8, EG_PAD] -> [128, F]: idx j of group g -> [16g + j%16, j//16]
        def wrap(r):
            return r.reshape(8, F, 16).transpose(0, 2, 1).reshape(128, F)

        in_maps.append({
            "embT": embT,
            "Wt": W_emb,
            "bias2": bias2,
            "wsym2": wsym2,
            "bedge": bedge,
            "iota16": iota16,
            "e8": e8,
            "r0w": np.ascontiguousarray(wrap(r0)),
            "r1w": np.ascontiguousarray(wrap(r1)),
            "c0u": np.ascontiguousarray(np.repeat(c0, 16, axis=0)),
            "c1u": np.ascontiguousarray(np.repeat(c1, 16, axis=0)),
            "uu": np.ascontiguousarray(up.reshape(128, F)),
        })
    return in_maps


def kernel(embedding, edges, u, W_emb, b_emb, W_edge, b_edge, _trace=False, _debug=False, _stage=5):
    key = (_debug, _stage)
    if key not in _PROGRAM_CACHE:
        _PROGRAM_CACHE[key] = _build_program(_debug, _stage)
    nc = _PROGRAM_CACHE[key]
    in_maps = _prep_inputs(embedding, edges, u, W_emb, b_emb, W_edge, b_edge)
    res = run_bass_kernel_spmd(nc, in_maps, core_ids=list(range(NCORES)), trace=_trace)
    outs = []
    for c in range(NCORES):
        o = res.results[c]["out"].reshape(8, EG_PAD)[:, :EG].reshape(-1)
        outs.append(o)
    full = np.concatenate(outs).astype(np.float32)
    if _debug or _trace:
        kernel._last_results = res
    return full
